# revision 10
# baseline (speedup 1.0000x reference)
"""CrossEncoderGNN (2x GIN layer + sum-pool + MLP + sigmoid) on 8 trn2 NeuronCores.

Strategy
--------
The two GIN layers have no nonlinearity ((h + Ah) @ W + b), and ReLU only
appears after pooling, so everything before the classifier is linear in x:

  pooled = B (I+A)^2 x W1 W2 + (B(I+A)1) (x) b1 W2 + n (x) b2
         = D @ x @ W1 @ W2 + outer(ne, b1 @ W2) + outer(n, b2)

with A[i,j] = #edges j->i, B[g,i] = [batch[i]==g], D = B(I+A)^2 a [64, 20000]
matrix of small integer path counts (host-computable from the edge list alone,
exact in f16), ne = B(I+A)1, n = graph sizes.

Device work per core (nodes split 8 ways, 2500 rows -> 20 tiles of 128):
  partial = sum_t D_t^T x_t          (20 f16 matmuls -> [64,512] PSUM)
  fold through W1 and W2 in transposed space (linear, so the AllReduce can
  happen after the weight applications): P2T = W2^T W1^T partial^T
  AllReduce P2T [128, 4x64] f16 (64 KB)
  tail: + rank-1 bias outers (computed on-device while the AR is in flight),
  classifier zT = relu(Wc1^T pooledT + bc1), score = Wc2^T zT, sigmoid.

Only integer count matrices / layout packs are built on the host; all float
math involving weights runs on device.
"""

import sys

for _p in ("/opt/trn_rl_repo", "/root/.axon_site/_ro/trn_rl_repo"):
    if _p not in sys.path:
        sys.path.insert(0, _p)

import os
import numpy as np

import concourse.bass as bass
import concourse.bacc as bacc
import concourse.tile as tile
from concourse import mybir
from concourse.bass_utils import run_bass_kernel_spmd
from concourse.masks import make_identity

N_NODES = 20000
N_EDGES = 320000
D = 512
N_GRAPHS = 64
N_CORES = 8
ROWS = N_NODES // N_CORES          # 2500 real rows per core
P = 128
TILES = (ROWS + P - 1) // P        # 20
KCH = D // P                       # 4 feature chunks of 128
XGRP = 5                           # x DMA granularity (tiles per chunk)
NXG = TILES // XGRP                # 4 x-chunks

LAST_EXEC_NS = None
LAST_RESULTS = None

_prog_cache = {}


def _build_program():
    f32 = mybir.dt.float32
    f16 = mybir.dt.float16

    nc = bacc.Bacc("TRN2", debug=False, num_devices=N_CORES, num_swdge_queues=4)

    # ---- I/O (per core) ----
    x_in = nc.dram_tensor("x_sh", [P, TILES * D], f16, kind="ExternalInput")
    dT_in = nc.dram_tensor("dT", [P, TILES * N_GRAPHS], f16, kind="ExternalInput")
    w1_in = nc.dram_tensor("w1", [P, KCH * D], f32, kind="ExternalInput")
    w2_in = nc.dram_tensor("w2", [P, KCH * D], f32, kind="ExternalInput")
    wc1_in = nc.dram_tensor("wc1", [P, KCH * 2 * P], f32, kind="ExternalInput")
    b1T_in = nc.dram_tensor("b1T", [P, KCH], f32, kind="ExternalInput")
    b2v_in = nc.dram_tensor("b2v", [1, D], f32, kind="ExternalInput")
    nev_in = nc.dram_tensor("nev", [1, N_GRAPHS], f32, kind="ExternalInput")
    ngv_in = nc.dram_tensor("ngv", [1, N_GRAPHS], f32, kind="ExternalInput")
    bc1_in = nc.dram_tensor("bc1", [P, 2], f32, kind="ExternalInput")
    wc2_in = nc.dram_tensor("wc2", [P, 2], f32, kind="ExternalInput")
    bc2_in = nc.dram_tensor("bc2", [1, 1], f32, kind="ExternalInput")
    scores = nc.dram_tensor("scores", [1, N_GRAPHS], f32, kind="ExternalOutput")

    # ---- internal DRAM ----
    ar_in = nc.dram_tensor("ar_in", [P, KCH * N_GRAPHS], f32)
    ar_out = nc.dram_tensor("ar_out", [P, KCH * N_GRAPHS], f32, addr_space="Shared")

    rg = [list(range(N_CORES))]

    with tile.TileContext(nc) as tc:
        with (
            tc.tile_pool(name="const", bufs=1) as const,
            tc.tile_pool(name="xbuf", bufs=1) as xbuf,
            tc.tile_pool(name="mlp", bufs=1) as mlp_pool,
            tc.tile_pool(name="ps", bufs=6, space="PSUM") as psA,
            tc.tile_pool(name="psAcc", bufs=1, space="PSUM") as psAcc,
        ):
            # x chunks alternate between the SP and POOL rings; constants
            # ride the ACT ring (small vectors first, then the weight packs
            # in the order the PE needs them).
            xv = x_in.ap().rearrange("p (t d) -> p t d", d=D)
            x_sb = []
            for g in range(NXG):
                xt = xbuf.tile([P, XGRP, D], f16, tag=f"x{g}")
                eng = nc.sync if g % 2 == 0 else nc.gpsimd
                eng.dma_start(out=xt[:], in_=xv[:, g * XGRP : (g + 1) * XGRP, :])
                x_sb.append(xt)

            dT_sb = const.tile([P, TILES * N_GRAPHS], f16)
            nc.scalar.dma_start(out=dT_sb[:], in_=dT_in[:])
            dT_v = dT_sb[:].rearrange("p (t g) -> p t g", g=N_GRAPHS)

            small = {}
            b1T_sb = const.tile([P, KCH], f32, name="c_b1T")
            nc.scalar.dma_start(out=b1T_sb[:], in_=b1T_in[:])
            for name, t_in, shp in (
                ("b2v", b2v_in, [1, D]),
                ("nev", nev_in, [1, N_GRAPHS]),
                ("ngv", ngv_in, [1, N_GRAPHS]),
                ("bc1", bc1_in, [P, 2]),
                ("wc2", wc2_in, [P, 2]),
                ("bc2", bc2_in, [1, 1]),
            ):
                st = const.tile(shp, f32, name=f"c_{name}")
                nc.scalar.dma_start(out=st[:], in_=t_in[:])
                small[name] = st
            w_sb = []
            for w_in in (w1_in, w2_in):
                wt = const.tile([P, KCH * D], f32)
                nc.scalar.dma_start(out=wt[:], in_=w_in[:])
                w_sb.append(wt[:].rearrange("p (j i) -> p j i", i=D))
            wc1_sb = const.tile([P, KCH * 2 * P], f32)
            nc.scalar.dma_start(out=wc1_sb[:], in_=wc1_in[:])
            wc1_v = wc1_sb[:].rearrange("p (j c m) -> p j c m", c=2, m=P)
            ident = const.tile([P, P], f32)
            make_identity(nc, ident[:])

            # ---- partial = D_local @ x_local : [64, 512] ----
            p0_ps = psAcc.tile([N_GRAPHS, D], f32)
            for t in range(TILES):
                nc.tensor.matmul(
                    out=p0_ps[:],
                    lhsT=dT_v[:, t, :],
                    rhs=x_sb[t // XGRP][:, t % XGRP, :],
                    start=(t == 0),
                    stop=(t == TILES - 1),
                    skip_group_check=True,
                )
            p0_sb = mlp_pool.tile([N_GRAPHS, D], f32)
            nc.vector.tensor_copy(out=p0_sb[:], in_=p0_ps[:])

            # ---- fold W1, W2 in transposed space (pre-AllReduce) ----
            p0T = mlp_pool.tile([P, KCH, N_GRAPHS], f32, name="p0T")
            for j in range(KCH):
                ps_t = psA.tile([P, N_GRAPHS], f32, tag="ps", name=f"p0T_{j}")
                nc.tensor.transpose(
                    out=ps_t[:],
                    in_=p0_sb[:, j * P : (j + 1) * P],
                    identity=ident[0:N_GRAPHS, 0:N_GRAPHS],
                )
                nc.vector.tensor_copy(out=p0T[:, j, :], in_=ps_t[:])

            def foldT(srcT, w_view, name):
                """outT[i] = sum_j W[j,i]^T srcT[j]  ([128,4,64] f16)."""
                out = mlp_pool.tile([P, KCH, N_GRAPHS], f32, name=name)
                for i in range(KCH):
                    ps = psA.tile([P, N_GRAPHS], f32, tag="ps", name=f"{name}_{i}")
                    for j in range(KCH):
                        nc.tensor.matmul(
                            out=ps[:],
                            lhsT=w_view[:, j, i * P : (i + 1) * P],
                            rhs=srcT[:, j, :],
                            start=(j == 0),
                            stop=(j == KCH - 1),
                        )
                    nc.vector.tensor_copy(out=out[:, i, :], in_=ps[:])
                return out

            p1T = foldT(p0T, w_sb[0], "p1T")
            p2T = foldT(p1T, w_sb[1], "p2T")

            p2T_flat = p2T[:].rearrange("p j g -> p (j g)")
            nc.sync.dma_start(out=ar_in[:], in_=p2T_flat)
            nc.gpsimd.collective_compute(
                "AllReduce", mybir.AluOpType.add, replica_groups=rg,
                ins=[ar_in[:]], outs=[ar_out[:]],
            )

            # ---- rank-1 bias outers, computed while the AR is in flight ----
            # pooled_biasT[i] = (b1 @ W2)^T_i (x) ne + b2^T_i (x) n
            u1_ps = psA.tile([1, D], f32, tag="ps", name="u1")
            for j in range(KCH):
                nc.tensor.matmul(
                    out=u1_ps[:],
                    lhsT=b1T_sb[:, j : j + 1],
                    rhs=w_sb[1][:, j, :],
                    start=(j == 0),
                    stop=(j == KCH - 1),
                )
            u1_sb = mlp_pool.tile([1, D], f32, name="u1sb")
            nc.vector.tensor_copy(out=u1_sb[:], in_=u1_ps[:])
            pbT = mlp_pool.tile([P, KCH, N_GRAPHS], f32, name="pbT")
            for i in range(KCH):
                ps = psA.tile([P, N_GRAPHS], f32, tag="ps", name=f"pb_{i}")
                nc.tensor.matmul(
                    out=ps[:],
                    lhsT=u1_sb[0:1, i * P : (i + 1) * P],
                    rhs=small["nev"][:],
                    start=True, stop=False,
                    skip_group_check=True,
                )
                nc.tensor.matmul(
                    out=ps[:],
                    lhsT=small["b2v"][0:1, i * P : (i + 1) * P],
                    rhs=small["ngv"][:],
                    start=False, stop=True,
                    skip_group_check=True,
                )
                nc.vector.tensor_copy(out=pbT[:, i, :], in_=ps[:])

            # ---- post-AR tail: bias add + classifier ----
            arT = mlp_pool.tile([P, KCH, N_GRAPHS], f32, name="arT")
            nc.sync.dma_start(
                out=arT[:].rearrange("p j g -> p (j g)"), in_=ar_out[:])
            pooledT = mlp_pool.tile([P, KCH, N_GRAPHS], f32, name="pooledT")
            nc.vector.tensor_add(out=pooledT[:], in0=arT[:], in1=pbT[:])

            zT = mlp_pool.tile([P, 2, N_GRAPHS], f32)
            for c2 in range(2):
                ps_z = psA.tile([P, N_GRAPHS], f32, tag="ps", name=f"mlp_z_{c2}")
                for j in range(KCH):
                    nc.tensor.matmul(
                        out=ps_z[:],
                        lhsT=wc1_v[:, j, c2, :],
                        rhs=pooledT[:, j, :],
                        start=(j == 0),
                        stop=(j == KCH - 1),
                    )
                nc.scalar.activation(
                    out=zT[:, c2, :], in_=ps_z[:],
                    func=mybir.ActivationFunctionType.Relu,
                    bias=small["bc1"][:, c2 : c2 + 1],
                )
            ps_s = psA.tile([1, N_GRAPHS], f32, tag="ps", name="mlp_s")
            for c2 in range(2):
                nc.tensor.matmul(
                    out=ps_s[:],
                    lhsT=small["wc2"][:, c2 : c2 + 1],
                    rhs=zT[:, c2, :],
                    start=(c2 == 0),
                    stop=(c2 == 1),
                )
            score_sb = mlp_pool.tile([1, N_GRAPHS], f32)
            nc.scalar.activation(
                out=score_sb[:], in_=ps_s[:],
                func=mybir.ActivationFunctionType.Sigmoid,
                bias=small["bc2"][0:1, 0:1],
            )
            nc.sync.dma_start(out=scores[:], in_=score_sb[:])

    nc.finalize()
    return nc


def _prep_inputs(joint_x, joint_edge_index, joint_batch,
                 W_g1, b_g1, W_g2, b_g2, W_c1, b_c1, W_c2, b_c2):
    import scipy.sparse as sp

    x = np.asarray(joint_x, np.float32)
    ei = np.asarray(joint_edge_index).astype(np.int64)
    batch = np.asarray(joint_batch).astype(np.int64)
    src, dst = ei[0], ei[1]

    # D = B (I+A)^2 : [64, 20000] integer path counts (exact in f16 if < 2048).
    ones = np.ones(N_EDGES, np.float64)
    A = sp.csr_matrix((ones, (dst, src)), shape=(N_NODES, N_NODES))
    M = sp.eye(N_NODES, format="csr") + A
    B = sp.csr_matrix(
        (np.ones(N_NODES, np.float64), (batch, np.arange(N_NODES))),
        shape=(N_GRAPHS, N_NODES),
    )
    C = np.asarray((B @ M).todense())              # [64, N] = B(I+A)
    D2 = M.T.dot(C.T).T                            # [64, N] = B(I+A)^2
    ne = C.sum(axis=1)                             # B(I+A)1 : n_g + E_g
    ng = np.bincount(batch, minlength=N_GRAPHS).astype(np.float64)

    F16 = np.float16
    w1_pack = np.ascontiguousarray(
        np.asarray(W_g1, np.float32).reshape(KCH, P, D)
        .transpose(1, 0, 2).reshape(P, KCH * D))
    w2_pack = np.ascontiguousarray(
        np.asarray(W_g2, np.float32).reshape(KCH, P, D)
        .transpose(1, 0, 2).reshape(P, KCH * D))
    wc1_pack = np.ascontiguousarray(
        np.asarray(W_c1, np.float32).reshape(KCH, P, 2, P)
        .transpose(1, 0, 2, 3).reshape(P, KCH * 2 * P))
    b1T_pack = np.ascontiguousarray(np.asarray(b_g1, np.float32).reshape(KCH, P).T)
    b2v_pack = np.asarray(b_g2, np.float32).reshape(1, D)
    nev_pack = ne.astype(np.float32).reshape(1, N_GRAPHS)
    ngv_pack = ng.astype(np.float32).reshape(1, N_GRAPHS)
    bc1_pack = np.ascontiguousarray(np.asarray(b_c1, np.float32).reshape(2, P).T)
    wc2_pack = np.ascontiguousarray(np.asarray(W_c2, np.float32).reshape(2, P).T)
    bc2_pack = np.asarray(b_c2, np.float32).reshape(1, 1)

    D2f = D2.astype(F16)
    in_maps = []
    for c in range(N_CORES):
        lo = c * ROWS
        xs = np.zeros((TILES * P, D), F16)
        xs[:ROWS] = x[lo : lo + ROWS]
        x_pack = np.ascontiguousarray(
            xs.reshape(TILES, P, D).transpose(1, 0, 2).reshape(P, TILES * D))

        dloc = np.zeros((N_GRAPHS, TILES * P), F16)
        dloc[:, :ROWS] = D2f[:, lo : lo + ROWS]
        dT_pack = np.ascontiguousarray(
            dloc.T.reshape(TILES, P, N_GRAPHS).transpose(1, 0, 2)
            .reshape(P, TILES * N_GRAPHS))

        in_maps.append({
            "x_sh": x_pack,
            "dT": dT_pack,
            "w1": w1_pack, "w2": w2_pack, "wc1": wc1_pack,
            "b1T": b1T_pack, "b2v": b2v_pack,
            "nev": nev_pack, "ngv": ngv_pack,
            "bc1": bc1_pack, "wc2": wc2_pack, "bc2": bc2_pack,
        })
    return in_maps


def kernel(**inputs):
    global LAST_EXEC_NS, LAST_RESULTS
    in_maps = _prep_inputs(**inputs)
    if "prog" not in _prog_cache:
        _prog_cache["prog"] = _build_program()
    nc = _prog_cache["prog"]
    trace = os.environ.get("GNN_TRACE", "0") == "1"
    res = run_bass_kernel_spmd(
        nc, in_maps, core_ids=list(range(N_CORES)), trace=trace,
        tmpdir=os.environ.get("GNN_TRACE_DIR") or None,
    )
    LAST_EXEC_NS = getattr(res, "exec_time_ns", None)
    LAST_RESULTS = res
    return np.asarray(res.results[0]["scores"]).reshape(N_GRAPHS).astype(np.float32)


# revision 11
# speedup vs baseline: 1.3686x; 1.3686x over previous
"""CrossEncoderGNN (2x GIN layer + sum-pool + MLP + sigmoid) on 8 trn2 NeuronCores.

Strategy
--------
The two GIN layers have no nonlinearity ((h + Ah) @ W + b), and ReLU only
appears after pooling, so everything before the classifier is linear in x:

  pooled = B (I+A)^2 x W1 W2 + (B(I+A)1) (x) b1 W2 + n (x) b2
         = D @ x @ W1 @ W2 + outer(ne, b1 @ W2) + outer(n, b2)

with A[i,j] = #edges j->i, B[g,i] = [batch[i]==g], D = B(I+A)^2 a [64, 20000]
matrix of small integer path counts (host-computable from the edge list alone,
exact in f16), ne = B(I+A)1, n = graph sizes.

Device work per core (nodes split 8 ways, 2500 rows -> 20 tiles of 128):
  partial = sum_t D_t^T x_t          (20 f16 matmuls -> [64,512] PSUM)
  fold through W1 and W2 in transposed space (linear, so the AllReduce can
  happen after the weight applications): P2T = W2^T W1^T partial^T
  AllReduce P2T [128, 4x64] f16 (64 KB)
  tail: + rank-1 bias outers (computed on-device while the AR is in flight),
  classifier zT = relu(Wc1^T pooledT + bc1), score = Wc2^T zT, sigmoid.

Only integer count matrices / layout packs are built on the host; all float
math involving weights runs on device.
"""

import sys

for _p in ("/opt/trn_rl_repo", "/root/.axon_site/_ro/trn_rl_repo"):
    if _p not in sys.path:
        sys.path.insert(0, _p)

import os
import numpy as np

import concourse.bass as bass
import concourse.bacc as bacc
import concourse.tile as tile
from concourse import mybir
from concourse.bass_utils import run_bass_kernel_spmd
from concourse.masks import make_identity

N_NODES = 20000
N_EDGES = 320000
D = 512
N_GRAPHS = 64
N_CORES = 8
ROWS = N_NODES // N_CORES          # 2500 real rows per core
P = 128
TILES = (ROWS + P - 1) // P        # 20
KCH = D // P                       # 4 feature chunks of 128
XGRP = 5                           # x DMA granularity (tiles per chunk)
NXG = TILES // XGRP                # 4 x-chunks

LAST_EXEC_NS = None
LAST_RESULTS = None

_prog_cache = {}


def _build_program():
    f32 = mybir.dt.float32
    f16 = mybir.dt.float16

    nc = bacc.Bacc("TRN2", debug=False, num_devices=N_CORES, num_swdge_queues=4)

    # ---- I/O (per core) ----
    x_in = nc.dram_tensor("x_sh", [P, TILES * D], f16, kind="ExternalInput")
    dT_in = nc.dram_tensor("dT", [P, TILES * N_GRAPHS], f16, kind="ExternalInput")
    w1_in = nc.dram_tensor("w1", [P, KCH * D], f32, kind="ExternalInput")
    w2_in = nc.dram_tensor("w2", [P, KCH * D], f32, kind="ExternalInput")
    wc1_in = nc.dram_tensor("wc1", [P, KCH * 2 * P], f32, kind="ExternalInput")
    b1T_in = nc.dram_tensor("b1T", [P, KCH], f32, kind="ExternalInput")
    b2v_in = nc.dram_tensor("b2v", [1, D], f32, kind="ExternalInput")
    nev_in = nc.dram_tensor("nev", [1, N_GRAPHS], f32, kind="ExternalInput")
    ngv_in = nc.dram_tensor("ngv", [1, N_GRAPHS], f32, kind="ExternalInput")
    bc1_in = nc.dram_tensor("bc1", [P, 2], f32, kind="ExternalInput")
    wc2_in = nc.dram_tensor("wc2", [P, 2], f32, kind="ExternalInput")
    bc2_in = nc.dram_tensor("bc2", [1, 1], f32, kind="ExternalInput")
    scores = nc.dram_tensor("scores", [1, N_GRAPHS], f32, kind="ExternalOutput")

    # ---- internal DRAM ----
    ar_in = nc.dram_tensor("ar_in", [P, 2 * N_GRAPHS], f32)
    ar_out = nc.dram_tensor("ar_out", [P, 2 * N_GRAPHS], f32, addr_space="Shared")

    rg = [list(range(N_CORES))]

    with tile.TileContext(nc) as tc:
        with (
            tc.tile_pool(name="const", bufs=1) as const,
            tc.tile_pool(name="xbuf", bufs=1) as xbuf,
            tc.tile_pool(name="mlp", bufs=1) as mlp_pool,
            tc.tile_pool(name="ps", bufs=6, space="PSUM") as psA,
            tc.tile_pool(name="psAcc", bufs=1, space="PSUM") as psAcc,
        ):
            # x chunks alternate between the SP and POOL rings; constants
            # ride the ACT ring (small vectors first, then the weight packs
            # in the order the PE needs them).
            xv = x_in.ap().rearrange("p (t d) -> p t d", d=D)
            x_sb = []
            for g in range(NXG):
                xt = xbuf.tile([P, XGRP, D], f16, tag=f"x{g}")
                eng = nc.sync if g % 2 == 0 else nc.gpsimd
                eng.dma_start(out=xt[:], in_=xv[:, g * XGRP : (g + 1) * XGRP, :])
                x_sb.append(xt)

            dT_sb = const.tile([P, TILES * N_GRAPHS], f16)
            nc.scalar.dma_start(out=dT_sb[:], in_=dT_in[:])
            dT_v = dT_sb[:].rearrange("p (t g) -> p t g", g=N_GRAPHS)

            small = {}
            b1T_sb = const.tile([P, KCH], f32, name="c_b1T")
            nc.scalar.dma_start(out=b1T_sb[:], in_=b1T_in[:])
            for name, t_in, shp in (
                ("b2v", b2v_in, [1, D]),
                ("nev", nev_in, [1, N_GRAPHS]),
                ("ngv", ngv_in, [1, N_GRAPHS]),
                ("bc1", bc1_in, [P, 2]),
                ("wc2", wc2_in, [P, 2]),
                ("bc2", bc2_in, [1, 1]),
            ):
                st = const.tile(shp, f32, name=f"c_{name}")
                nc.scalar.dma_start(out=st[:], in_=t_in[:])
                small[name] = st
            w_sb = []
            for w_in in (w1_in, w2_in):
                wt = const.tile([P, KCH * D], f32)
                nc.scalar.dma_start(out=wt[:], in_=w_in[:])
                w_sb.append(wt[:].rearrange("p (j i) -> p j i", i=D))
            wc1_sb = const.tile([P, KCH * 2 * P], f32)
            nc.scalar.dma_start(out=wc1_sb[:], in_=wc1_in[:])
            wc1_v = wc1_sb[:].rearrange("p (j c m) -> p j c m", c=2, m=P)
            ident = const.tile([P, P], f32)
            make_identity(nc, ident[:])

            # ---- partial = D_local @ x_local : [64, 512] ----
            p0_ps = psAcc.tile([N_GRAPHS, D], f32)
            for t in range(TILES):
                nc.tensor.matmul(
                    out=p0_ps[:],
                    lhsT=dT_v[:, t, :],
                    rhs=x_sb[t // XGRP][:, t % XGRP, :],
                    start=(t == 0),
                    stop=(t == TILES - 1),
                    skip_group_check=True,
                )
            p0_sb = mlp_pool.tile([N_GRAPHS, D], f32)
            nc.vector.tensor_copy(out=p0_sb[:], in_=p0_ps[:])

            # ---- fold W1, W2 in transposed space (pre-AllReduce) ----
            p0T = mlp_pool.tile([P, KCH, N_GRAPHS], f32, name="p0T")
            for j in range(KCH):
                ps_t = psA.tile([P, N_GRAPHS], f32, tag="ps", name=f"p0T_{j}")
                nc.tensor.transpose(
                    out=ps_t[:],
                    in_=p0_sb[:, j * P : (j + 1) * P],
                    identity=ident[0:N_GRAPHS, 0:N_GRAPHS],
                )
                nc.vector.tensor_copy(out=p0T[:, j, :], in_=ps_t[:])

            def foldT(srcT, w_view, name):
                """outT[i] = sum_j W[j,i]^T srcT[j]  ([128,4,64] f16)."""
                out = mlp_pool.tile([P, KCH, N_GRAPHS], f32, name=name)
                for i in range(KCH):
                    ps = psA.tile([P, N_GRAPHS], f32, tag="ps", name=f"{name}_{i}")
                    for j in range(KCH):
                        nc.tensor.matmul(
                            out=ps[:],
                            lhsT=w_view[:, j, i * P : (i + 1) * P],
                            rhs=srcT[:, j, :],
                            start=(j == 0),
                            stop=(j == KCH - 1),
                        )
                    nc.vector.tensor_copy(out=out[:, i, :], in_=ps[:])
                return out

            p1T = foldT(p0T, w_sb[0], "p1T")
            p2T = foldT(p1T, w_sb[1], "p2T")

            # fold Wc1 as well: the AR carries z-pre [128, 2, 64] (64 KB)
            def foldC(srcT, name):
                out = mlp_pool.tile([P, 2, N_GRAPHS], f32, name=name)
                for c2 in range(2):
                    ps = psA.tile([P, N_GRAPHS], f32, tag="ps", name=f"{name}_{c2}")
                    for j in range(KCH):
                        nc.tensor.matmul(
                            out=ps[:],
                            lhsT=wc1_v[:, j, c2, :],
                            rhs=srcT[:, j, :],
                            start=(j == 0),
                            stop=(j == KCH - 1),
                        )
                    nc.vector.tensor_copy(out=out[:, c2, :], in_=ps[:])
                return out

            zpreT = foldC(p2T, "zpreT")
            nc.sync.dma_start(
                out=ar_in[:], in_=zpreT[:].rearrange("p c g -> p (c g)"))
            nc.gpsimd.collective_compute(
                "AllReduce", mybir.AluOpType.add, replica_groups=rg,
                ins=[ar_in[:]], outs=[ar_out[:]],
            )

            # ---- rank-1 bias outers, computed while the AR is in flight ----
            # pooled_biasT[i] = (b1 @ W2)^T_i (x) ne + b2^T_i (x) n
            u1_ps = psA.tile([1, D], f32, tag="ps", name="u1")
            for j in range(KCH):
                nc.tensor.matmul(
                    out=u1_ps[:],
                    lhsT=b1T_sb[:, j : j + 1],
                    rhs=w_sb[1][:, j, :],
                    start=(j == 0),
                    stop=(j == KCH - 1),
                )
            u1_sb = mlp_pool.tile([1, D], f32, name="u1sb")
            nc.vector.tensor_copy(out=u1_sb[:], in_=u1_ps[:])
            pbT = mlp_pool.tile([P, KCH, N_GRAPHS], f32, name="pbT")
            for i in range(KCH):
                ps = psA.tile([P, N_GRAPHS], f32, tag="ps", name=f"pb_{i}")
                nc.tensor.matmul(
                    out=ps[:],
                    lhsT=u1_sb[0:1, i * P : (i + 1) * P],
                    rhs=small["nev"][:],
                    start=True, stop=False,
                    skip_group_check=True,
                )
                nc.tensor.matmul(
                    out=ps[:],
                    lhsT=small["b2v"][0:1, i * P : (i + 1) * P],
                    rhs=small["ngv"][:],
                    start=False, stop=True,
                    skip_group_check=True,
                )
                nc.vector.tensor_copy(out=pbT[:, i, :], in_=ps[:])

            # bias in z-space (also while the AR is in flight)
            zbT = foldC(pbT, "zbT")

            # ---- post-AR tail: bias add + ReLU + score ----
            arT = mlp_pool.tile([P, 2, N_GRAPHS], f32, name="arT")
            nc.sync.dma_start(
                out=arT[:].rearrange("p c g -> p (c g)"), in_=ar_out[:])
            zsT = mlp_pool.tile([P, 2, N_GRAPHS], f32, name="zsT")
            nc.vector.tensor_add(out=zsT[:], in0=arT[:], in1=zbT[:])

            zT = mlp_pool.tile([P, 2, N_GRAPHS], f32)
            for c2 in range(2):
                nc.scalar.activation(
                    out=zT[:, c2, :], in_=zsT[:, c2, :],
                    func=mybir.ActivationFunctionType.Relu,
                    bias=small["bc1"][:, c2 : c2 + 1],
                )
            ps_s = psA.tile([1, N_GRAPHS], f32, tag="ps", name="mlp_s")
            for c2 in range(2):
                nc.tensor.matmul(
                    out=ps_s[:],
                    lhsT=small["wc2"][:, c2 : c2 + 1],
                    rhs=zT[:, c2, :],
                    start=(c2 == 0),
                    stop=(c2 == 1),
                )
            score_sb = mlp_pool.tile([1, N_GRAPHS], f32)
            nc.scalar.activation(
                out=score_sb[:], in_=ps_s[:],
                func=mybir.ActivationFunctionType.Sigmoid,
                bias=small["bc2"][0:1, 0:1],
            )
            nc.sync.dma_start(out=scores[:], in_=score_sb[:])

    nc.finalize()
    return nc


def _prep_inputs(joint_x, joint_edge_index, joint_batch,
                 W_g1, b_g1, W_g2, b_g2, W_c1, b_c1, W_c2, b_c2):
    import scipy.sparse as sp

    x = np.asarray(joint_x, np.float32)
    ei = np.asarray(joint_edge_index).astype(np.int64)
    batch = np.asarray(joint_batch).astype(np.int64)
    src, dst = ei[0], ei[1]

    # D = B (I+A)^2 : [64, 20000] integer path counts (exact in f16 if < 2048).
    ones = np.ones(N_EDGES, np.float64)
    A = sp.csr_matrix((ones, (dst, src)), shape=(N_NODES, N_NODES))
    M = sp.eye(N_NODES, format="csr") + A
    B = sp.csr_matrix(
        (np.ones(N_NODES, np.float64), (batch, np.arange(N_NODES))),
        shape=(N_GRAPHS, N_NODES),
    )
    C = np.asarray((B @ M).todense())              # [64, N] = B(I+A)
    D2 = M.T.dot(C.T).T                            # [64, N] = B(I+A)^2
    ne = C.sum(axis=1)                             # B(I+A)1 : n_g + E_g
    ng = np.bincount(batch, minlength=N_GRAPHS).astype(np.float64)

    F16 = np.float16
    w1_pack = np.ascontiguousarray(
        np.asarray(W_g1, np.float32).reshape(KCH, P, D)
        .transpose(1, 0, 2).reshape(P, KCH * D))
    w2_pack = np.ascontiguousarray(
        np.asarray(W_g2, np.float32).reshape(KCH, P, D)
        .transpose(1, 0, 2).reshape(P, KCH * D))
    wc1_pack = np.ascontiguousarray(
        np.asarray(W_c1, np.float32).reshape(KCH, P, 2, P)
        .transpose(1, 0, 2, 3).reshape(P, KCH * 2 * P))
    b1T_pack = np.ascontiguousarray(np.asarray(b_g1, np.float32).reshape(KCH, P).T)
    b2v_pack = np.asarray(b_g2, np.float32).reshape(1, D)
    nev_pack = ne.astype(np.float32).reshape(1, N_GRAPHS)
    ngv_pack = ng.astype(np.float32).reshape(1, N_GRAPHS)
    bc1_pack = np.ascontiguousarray(np.asarray(b_c1, np.float32).reshape(2, P).T)
    wc2_pack = np.ascontiguousarray(np.asarray(W_c2, np.float32).reshape(2, P).T)
    bc2_pack = np.asarray(b_c2, np.float32).reshape(1, 1)

    D2f = D2.astype(F16)
    in_maps = []
    for c in range(N_CORES):
        lo = c * ROWS
        xs = np.zeros((TILES * P, D), F16)
        xs[:ROWS] = x[lo : lo + ROWS]
        x_pack = np.ascontiguousarray(
            xs.reshape(TILES, P, D).transpose(1, 0, 2).reshape(P, TILES * D))

        dloc = np.zeros((N_GRAPHS, TILES * P), F16)
        dloc[:, :ROWS] = D2f[:, lo : lo + ROWS]
        dT_pack = np.ascontiguousarray(
            dloc.T.reshape(TILES, P, N_GRAPHS).transpose(1, 0, 2)
            .reshape(P, TILES * N_GRAPHS))

        in_maps.append({
            "x_sh": x_pack,
            "dT": dT_pack,
            "w1": w1_pack, "w2": w2_pack, "wc1": wc1_pack,
            "b1T": b1T_pack, "b2v": b2v_pack,
            "nev": nev_pack, "ngv": ngv_pack,
            "bc1": bc1_pack, "wc2": wc2_pack, "bc2": bc2_pack,
        })
    return in_maps


def kernel(**inputs):
    global LAST_EXEC_NS, LAST_RESULTS
    in_maps = _prep_inputs(**inputs)
    if "prog" not in _prog_cache:
        _prog_cache["prog"] = _build_program()
    nc = _prog_cache["prog"]
    trace = os.environ.get("GNN_TRACE", "0") == "1"
    res = run_bass_kernel_spmd(
        nc, in_maps, core_ids=list(range(N_CORES)), trace=trace,
        tmpdir=os.environ.get("GNN_TRACE_DIR") or None,
    )
    LAST_EXEC_NS = getattr(res, "exec_time_ns", None)
    LAST_RESULTS = res
    return np.asarray(res.results[0]["scores"]).reshape(N_GRAPHS).astype(np.float32)
